# revision 1
# baseline (speedup 1.0000x reference)
"""LlamaSkipMLP Trainium2 kernel.

Strategy: data-parallel over the token dim across 8 NeuronCores (no
collectives).  Each core computes out_c = silu(x_c@Wg'.T) * (x_c@Wu'.T) @ Wd'.T
for its 1024-token slice, where Wg'/Wu'/Wd' are the active-neuron
gather of the weights (done host-side; for active_idx = arange(k) it
is a plain slice).

Device kernel (per core, Tile framework):
  phase 1: g/u GEMMs contract hidden dim H (on PE partitions), fused
           SiLU*up on ACT+DVE, h stored [k_part, t_free] in fp16.
  phase 2: down GEMM contracts the active-neuron dim k; h tiles serve
           as the stationary operand, W_down^T tiles as the moving
           operand, so the output lands as [t_part, h_free] and stores
           contiguously.

All matmuls run in fp16 (PSUM accumulates fp32).  Host pre-lays-out
weights/activations so every DMA is contiguous and no on-device
transposes are needed.
"""

import numpy as np

# Problem shapes (hardcoded per spec).
T, H, K = 8192, 4096, 3302
NCORES = 8
KP = 3328                 # K padded to a multiple of 128
NK0 = KP // 128           # 26 k-tiles
NH0 = H // 128            # 32 h-tiles (contraction, phase 1)
TC = T // NCORES          # 1024 tokens per core

_CACHE = {}


def build_nc(kp=KP, h=H, tct=TC, enable_asserts=False):
    """Build + compile the per-core Bass program (SPMD: same on all cores)."""
    from contextlib import ExitStack

    import concourse.mybir as mybir
    import concourse.tile as tile
    from concourse import bacc

    fp16 = mybir.dt.float16
    fp32 = mybir.dt.float32
    Sigmoid = mybir.ActivationFunctionType.Sigmoid

    nk0 = kp // 128
    nh0 = h // 128
    ntf = tct // 512          # moving t-tiles, phase 1
    nt1 = tct // 128          # stationary t-tiles, phase 2
    nhf = h // 512            # moving h-tiles, phase 2

    nc = bacc.Bacc(
        "TRN2", target_bir_lowering=False, debug=False,
        enable_asserts=enable_asserts,
    )
    xt = nc.dram_tensor("xt", [128, nh0 * tct], fp16, kind="ExternalInput").ap()
    wg = nc.dram_tensor("wg", [nk0, 128, nh0 * 128], fp16, kind="ExternalInput").ap()
    wu = nc.dram_tensor("wu", [nk0, 128, nh0 * 128], fp16, kind="ExternalInput").ap()
    wd = nc.dram_tensor("wd", [nk0, 128, h], fp16, kind="ExternalInput").ap()
    out = nc.dram_tensor("out", [tct, h], fp32, kind="ExternalOutput").ap()

    with tile.TileContext(nc) as tc, ExitStack() as ctx:
        xt_pool = ctx.enter_context(tc.tile_pool(name="xtp", bufs=1))
        w_pool = ctx.enter_context(tc.tile_pool(name="wp", bufs=3))
        wd_pool = ctx.enter_context(tc.tile_pool(name="wdp", bufs=8))
        h_pool = ctx.enter_context(tc.tile_pool(name="hp", bufs=1))
        tmp_pool = ctx.enter_context(tc.tile_pool(name="tmpp", bufs=4))
        out_pool = ctx.enter_context(tc.tile_pool(name="outp", bufs=8))

        xt_sb = xt_pool.tile([128, nh0 * tct], fp16, name="xt_sb")
        h_sb = h_pool.tile([128, nk0 * tct], fp16, name="h_sb")

        # First k0's gate slab goes ahead of the x^T load so the first
        # matmul's stationary operand lands quickly; the up slab is only
        # needed ~13us later (after the 64 gate matmuls) so it follows
        # the first x^T chunks.
        wg_t0 = w_pool.tile([128, nh0 * 128], fp16, name="wg_t", tag="wg")
        nc.sync.dma_start(wg_t0[:, :], wg[0])
        # x^T in fine chunks: the h0=0 matmul only needs the first chunk.
        nchunk = max(1, (nh0 * tct) // 2048)
        csz = nh0 * tct // nchunk
        for i in range(nchunk):
            nc.sync.dma_start(xt_sb[:, i * csz:(i + 1) * csz],
                              xt[:, i * csz:(i + 1) * csz])
            if i == 1:
                wu_t0 = w_pool.tile([128, nh0 * 128], fp16, name="wu_t", tag="wu")
                nc.sync.dma_start(wu_t0[:, :], wu[0])
        if nchunk <= 1:
            wu_t0 = w_pool.tile([128, nh0 * 128], fp16, name="wu_t", tag="wu")
            nc.sync.dma_start(wu_t0[:, :], wu[0])

        # ---- phase 1: g = x@Wg^T, u = x@Wu^T, h = silu(g)*u ----
        with tc.tile_pool(name="ps1", space="PSUM", bufs=2) as ps1:
            for k0 in range(nk0):
                if k0 == 0:
                    wg_t, wu_t = wg_t0, wu_t0
                else:
                    wg_t = w_pool.tile([128, nh0 * 128], fp16, name="wg_t", tag="wg")
                    nc.sync.dma_start(wg_t[:, :], wg[k0])
                    wu_t = w_pool.tile([128, nh0 * 128], fp16, name="wu_t", tag="wu")
                    nc.sync.dma_start(wu_t[:, :], wu[k0])
                pg = [ps1.tile([128, 512], fp32, name=f"pg{i}", tag=f"pg{i}")
                      for i in range(ntf)]
                pu = [ps1.tile([128, 512], fp32, name=f"pu{i}", tag=f"pu{i}")
                      for i in range(ntf)]
                for h0 in range(nh0):
                    for i in range(ntf):
                        nc.tensor.matmul(
                            pg[i][:, :], wg_t[:, h0 * 128:(h0 + 1) * 128],
                            xt_sb[:, h0 * tct + i * 512:h0 * tct + (i + 1) * 512],
                            start=(h0 == 0), stop=(h0 == nh0 - 1),
                        )
                for h0 in range(nh0):
                    for i in range(ntf):
                        nc.tensor.matmul(
                            pu[i][:, :], wu_t[:, h0 * 128:(h0 + 1) * 128],
                            xt_sb[:, h0 * tct + i * 512:h0 * tct + (i + 1) * 512],
                            start=(h0 == 0), stop=(h0 == nh0 - 1),
                        )
                for i in range(ntf):
                    sg = tmp_pool.tile([128, 512], fp32, name="sg", tag="sg")
                    nc.scalar.activation(sg[:, :], pg[i][:, :], Sigmoid)
                    sl = tmp_pool.tile([128, 512], fp32, name="sl", tag="sl")
                    nc.vector.tensor_mul(sl[:, :], sg[:, :], pg[i][:, :])
                    nc.vector.tensor_mul(
                        h_sb[:, k0 * tct + i * 512:k0 * tct + (i + 1) * 512],
                        sl[:, :], pu[i][:, :])

        # ---- phase 2: out = h @ Wd^T (contract k) ----
        Copy = mybir.ActivationFunctionType.Copy
        with tc.tile_pool(name="ps2", space="PSUM", bufs=1) as ps2:
            for hf in range(nhf):
                po = [ps2.tile([128, 512], fp32, name=f"po{t1}", tag=f"po{t1}")
                      for t1 in range(nt1)]
                for k0 in range(nk0):
                    wd_t = wd_pool.tile([128, 512], fp16, name="wd_t", tag="wd")
                    nc.sync.dma_start(wd_t[:, :], wd[k0, :, hf * 512:(hf + 1) * 512])
                    for t1 in range(nt1):
                        nc.tensor.matmul(
                            po[t1][:, :],
                            h_sb[:, k0 * tct + t1 * 128:k0 * tct + (t1 + 1) * 128],
                            wd_t[:, :],
                            start=(k0 == 0), stop=(k0 == nk0 - 1),
                        )
                # Drains alternate DVE / ACT so the two engines empty the
                # PSUM banks in parallel and the next hf's matmuls don't
                # stall on bank reuse.
                for t1 in range(nt1):
                    ot = out_pool.tile([128, 512], fp32, name="ot", tag="ot")
                    if t1 % 2 == 0:
                        nc.vector.tensor_copy(ot[:, :], po[t1][:, :])
                    else:
                        nc.scalar.activation(ot[:, :], po[t1][:, :], Copy)
                    nc.sync.dma_start(
                        out[t1 * 128:(t1 + 1) * 128, hf * 512:(hf + 1) * 512],
                        ot[:, :])

    nc.compile()
    return nc


def prep_weights(W_gate, W_up, W_down, active_idx, kp=KP, h=H):
    idx = np.asarray(active_idx)
    k = idx.shape[0]
    nk0 = kp // 128
    nh0 = h // 128

    def lay_gu(W):
        a = np.zeros((kp, h), np.float16)
        a[:k] = W[idx].astype(np.float16)
        # [k0, p, h0*128 + k_in] = a[k0*128+k_in, h0*128+p]
        return np.ascontiguousarray(
            a.reshape(nk0, 128, nh0, 128).transpose(0, 3, 2, 1)
        ).reshape(nk0, 128, nh0 * 128)

    wd_a = np.zeros((kp, h), np.float16)
    wd_a[:k] = W_down[:, idx].T.astype(np.float16)
    wd_prep = np.ascontiguousarray(wd_a.reshape(nk0, 128, h))
    return lay_gu(W_gate), lay_gu(W_up), wd_prep


def prep_x_core(xc, h=H, tct=TC):
    nh0 = h // 128
    xt_c = np.ascontiguousarray(
        xc.astype(np.float16).T.reshape(nh0, 128, tct).transpose(1, 0, 2))
    return xt_c.reshape(128, nh0 * tct)


def run(inputs, trace=False, **kw):
    from concourse.bass_utils import run_bass_kernel_spmd

    if "nc" not in _CACHE:
        _CACHE["nc"] = build_nc()
    nc = _CACHE["nc"]

    wg_prep, wu_prep, wd_prep = prep_weights(
        inputs["W_gate"], inputs["W_up"], inputs["W_down"], inputs["active_idx"])
    x = inputs["x"]
    in_maps = [
        {"xt": prep_x_core(x[c * TC:(c + 1) * TC]),
         "wg": wg_prep, "wu": wu_prep, "wd": wd_prep}
        for c in range(NCORES)
    ]
    res = run_bass_kernel_spmd(nc, in_maps, core_ids=list(range(NCORES)),
                               trace=trace, **kw)
    out = np.concatenate([res.results[c]["out"] for c in range(NCORES)], axis=0)
    return out, res


def kernel(**inputs):
    out, _ = run(inputs, trace=False)
    return out



# revision 2
# speedup vs baseline: 1.1926x; 1.1926x over previous
"""LlamaSkipMLP Trainium2 kernel.

Strategy: data-parallel over the token dim across 8 NeuronCores (no
collectives).  Each core computes out_c = silu(x_c@Wg'.T) * (x_c@Wu'.T) @ Wd'.T
for its 1024-token slice, where Wg'/Wu'/Wd' are the active-neuron
gather of the weights (done host-side; for active_idx = arange(k) it
is a plain slice).

Device kernel (per core, Tile framework):
  phase 1: g/u GEMMs contract hidden dim H (on PE partitions).  The
           gate and up matmuls are interleaved within a single h0
           sweep per k0 so the x^T DMA only has to sustain ~300GB/s
           during k0=0 (instead of ~600GB/s for split sweeps), which
           lets the PE start ~13us earlier.  SiLU on ACT, h=silu*up
           on DVE, h stored [k_part, t_free] in fp16.
  phase 2: down GEMM contracts the active-neuron dim k; h tiles are
           the stationary operand, W_down^T tiles the moving operand,
           so the output lands as [t_part, h_free] and stores
           contiguously.  The last hf block runs t1-outer/k0-inner
           against SBUF-resident wd tiles so its 8 PSUM groups finish
           staggered and the final drain+store tail is ~2us instead
           of ~17us.

All matmuls run in fp16 (PSUM accumulates fp32).  Host pre-lays-out
weights/activations so every DMA is contiguous and no on-device
transposes are needed.
"""

import numpy as np

# Problem shapes (hardcoded per spec).
T, H, K = 8192, 4096, 3302
NCORES = 8
KP = 3328                 # K padded to a multiple of 128
NK0 = KP // 128           # 26 k-tiles
NH0 = H // 128            # 32 h-tiles (contraction, phase 1)
TC = T // NCORES          # 1024 tokens per core

_CACHE = {}


def build_nc(kp=KP, h=H, tct=TC, enable_asserts=False):
    """Build + compile the per-core Bass program (SPMD: same on all cores)."""
    from contextlib import ExitStack

    import concourse.mybir as mybir
    import concourse.tile as tile
    from concourse import bacc

    fp16 = mybir.dt.float16
    fp32 = mybir.dt.float32
    Silu = mybir.ActivationFunctionType.Silu
    Copy = mybir.ActivationFunctionType.Copy

    nk0 = kp // 128
    nh0 = h // 128
    ntf = tct // 512          # moving t-tiles, phase 1 (2)
    nt1 = tct // 128          # stationary t-tiles, phase 2 (8)
    nhf = h // 512            # moving h-tiles, phase 2 (8)

    nc = bacc.Bacc(
        "TRN2", target_bir_lowering=False, debug=False,
        enable_asserts=enable_asserts,
    )
    xt = nc.dram_tensor("xt", [128, nh0 * tct], fp16, kind="ExternalInput").ap()
    wg = nc.dram_tensor("wg", [nk0, 128, nh0 * 128], fp16, kind="ExternalInput").ap()
    wu = nc.dram_tensor("wu", [nk0, 128, nh0 * 128], fp16, kind="ExternalInput").ap()
    wd = nc.dram_tensor("wd", [nk0, 128, h], fp16, kind="ExternalInput").ap()
    out = nc.dram_tensor("out", [tct, h], fp32, kind="ExternalOutput").ap()

    with tile.TileContext(nc) as tc, ExitStack() as ctx:
        h_pool = ctx.enter_context(tc.tile_pool(name="hp", bufs=1))
        w_pool = ctx.enter_context(tc.tile_pool(name="wp", bufs=2))
        tmp_pool = ctx.enter_context(tc.tile_pool(name="tmpp", bufs=2))
        out_pool = ctx.enter_context(tc.tile_pool(name="outp", bufs=4))
        wd7_pool = ctx.enter_context(tc.tile_pool(name="wd7p", bufs=1))
        wd_pool = ctx.enter_context(tc.tile_pool(name="wdp", bufs=8))
        xt_pool = ctx.enter_context(tc.tile_pool(name="xtp", bufs=1))

        xt_sb = xt_pool.tile([128, nh0 * tct], fp16, name="xt_sb")
        h_sb = h_pool.tile([128, nk0 * tct], fp16, name="h_sb")

        # Resident wd tiles for the last hf block (loaded leisurely
        # during phase 1, one per k0 iteration).
        wd7_t = [wd7_pool.tile([128, 512], fp16, name=f"wd7_{k}", tag=f"wd7_{k}")
                 for k in range(nk0)]

        # k0=0 weight slabs in 4 chunks each, interleaved with the
        # leading x^T chunks so the first matmul can start ~1us in.
        wg_t0 = w_pool.tile([128, nh0 * 128], fp16, name="wg_t", tag="wg")
        wu_t0 = w_pool.tile([128, nh0 * 128], fp16, name="wu_t", tag="wu")
        wcsz = nh0 * 128 // 4
        xcsz = 1024                     # x^T chunk cols (256KB)
        nxchunk = nh0 * tct // xcsz
        nc.sync.dma_start(wg_t0[:, 0:wcsz], wg[0, :, 0:wcsz])
        nc.sync.dma_start(wu_t0[:, 0:wcsz], wu[0, :, 0:wcsz])
        nc.sync.dma_start(xt_sb[:, 0:xcsz], xt[:, 0:xcsz])
        for i in range(1, 4):
            nc.sync.dma_start(xt_sb[:, i * xcsz:(i + 1) * xcsz],
                              xt[:, i * xcsz:(i + 1) * xcsz])
            nc.sync.dma_start(wg_t0[:, i * wcsz:(i + 1) * wcsz],
                              wg[0, :, i * wcsz:(i + 1) * wcsz])
            nc.sync.dma_start(wu_t0[:, i * wcsz:(i + 1) * wcsz],
                              wu[0, :, i * wcsz:(i + 1) * wcsz])
        for i in range(4, nxchunk):
            nc.sync.dma_start(xt_sb[:, i * xcsz:(i + 1) * xcsz],
                              xt[:, i * xcsz:(i + 1) * xcsz])

        # ---- phase 1: g = x@Wg^T, u = x@Wu^T, h = silu(g)*u ----
        with tc.tile_pool(name="ps1", space="PSUM", bufs=2) as ps1:
            for k0 in range(nk0):
                if k0 == 0:
                    wg_t, wu_t = wg_t0, wu_t0
                else:
                    wg_t = w_pool.tile([128, nh0 * 128], fp16, name="wg_t", tag="wg")
                    nc.sync.dma_start(wg_t[:, :], wg[k0])
                    wu_t = w_pool.tile([128, nh0 * 128], fp16, name="wu_t", tag="wu")
                    nc.sync.dma_start(wu_t[:, :], wu[k0])
                # Spread the resident last-hf wd loads across phase 1.
                nc.sync.dma_start(wd7_t[k0][:, :],
                                  wd[k0, :, (nhf - 1) * 512:nhf * 512])
                pg = [ps1.tile([128, 512], fp32, name=f"pg{i}", tag=f"pg{i}")
                      for i in range(ntf)]
                pu = [ps1.tile([128, 512], fp32, name=f"pu{i}", tag=f"pu{i}")
                      for i in range(ntf)]
                # Interleave gate/up so each x chunk feeds 4 matmuls.
                for h0 in range(nh0):
                    for i in range(ntf):
                        nc.tensor.matmul(
                            pg[i][:, :], wg_t[:, h0 * 128:(h0 + 1) * 128],
                            xt_sb[:, h0 * tct + i * 512:h0 * tct + (i + 1) * 512],
                            start=(h0 == 0), stop=(h0 == nh0 - 1),
                        )
                        nc.tensor.matmul(
                            pu[i][:, :], wu_t[:, h0 * 128:(h0 + 1) * 128],
                            xt_sb[:, h0 * tct + i * 512:h0 * tct + (i + 1) * 512],
                            start=(h0 == 0), stop=(h0 == nh0 - 1),
                        )
                for i in range(ntf):
                    sg = tmp_pool.tile([128, 512], fp32, name="sg", tag="sg")
                    nc.scalar.activation(sg[:, :], pg[i][:, :], Silu)
                    nc.vector.tensor_mul(
                        h_sb[:, k0 * tct + i * 512:k0 * tct + (i + 1) * 512],
                        sg[:, :], pu[i][:, :])

        # ---- phase 2: out = h @ Wd^T (contract k) ----
        with tc.tile_pool(name="ps2", space="PSUM", bufs=1) as ps2:
            for hf in range(nhf - 1):
                po = [ps2.tile([128, 512], fp32, name=f"po{t1}", tag=f"po{t1}")
                      for t1 in range(nt1)]
                for k0 in range(nk0):
                    wd_t = wd_pool.tile([128, 512], fp16, name="wd_t", tag="wd")
                    nc.sync.dma_start(wd_t[:, :], wd[k0, :, hf * 512:(hf + 1) * 512])
                    for t1 in range(nt1):
                        nc.tensor.matmul(
                            po[t1][:, :],
                            h_sb[:, k0 * tct + t1 * 128:k0 * tct + (t1 + 1) * 128],
                            wd_t[:, :],
                            start=(k0 == 0), stop=(k0 == nk0 - 1),
                        )
                # Drains alternate DVE / ACT so the two engines empty the
                # PSUM banks in parallel and the next hf's matmuls don't
                # stall on bank reuse.
                for t1 in range(nt1):
                    ot = out_pool.tile([128, 512], fp32, name="ot", tag="ot")
                    if t1 % 2 == 0:
                        nc.vector.tensor_copy(ot[:, :], po[t1][:, :])
                    else:
                        nc.scalar.activation(ot[:, :], po[t1][:, :], Copy)
                    nc.sync.dma_start(
                        out[t1 * 128:(t1 + 1) * 128, hf * 512:(hf + 1) * 512],
                        ot[:, :])
            # Last hf: t1-outer / k0-inner against resident wd tiles, so
            # each PSUM group completes 26 matmuls before the next starts
            # and drains+stores overlap the remaining matmuls.
            hf = nhf - 1
            for t1 in range(nt1):
                po = ps2.tile([128, 512], fp32, name=f"po{t1}", tag=f"po{t1}")
                for k0 in range(nk0):
                    nc.tensor.matmul(
                        po[:, :],
                        h_sb[:, k0 * tct + t1 * 128:k0 * tct + (t1 + 1) * 128],
                        wd7_t[k0][:, :],
                        start=(k0 == 0), stop=(k0 == nk0 - 1),
                    )
                ot = out_pool.tile([128, 512], fp32, name="ot", tag="ot")
                if t1 % 2 == 0:
                    nc.vector.tensor_copy(ot[:, :], po[:, :])
                else:
                    nc.scalar.activation(ot[:, :], po[:, :], Copy)
                nc.sync.dma_start(
                    out[t1 * 128:(t1 + 1) * 128, hf * 512:(hf + 1) * 512],
                    ot[:, :])

    nc.compile()
    return nc


def prep_weights(W_gate, W_up, W_down, active_idx, kp=KP, h=H):
    idx = np.asarray(active_idx)
    k = idx.shape[0]
    nk0 = kp // 128
    nh0 = h // 128

    def lay_gu(W):
        a = np.zeros((kp, h), np.float16)
        a[:k] = W[idx].astype(np.float16)
        # [k0, p, h0*128 + k_in] = a[k0*128+k_in, h0*128+p]
        return np.ascontiguousarray(
            a.reshape(nk0, 128, nh0, 128).transpose(0, 3, 2, 1)
        ).reshape(nk0, 128, nh0 * 128)

    wd_a = np.zeros((kp, h), np.float16)
    wd_a[:k] = W_down[:, idx].T.astype(np.float16)
    wd_prep = np.ascontiguousarray(wd_a.reshape(nk0, 128, h))
    return lay_gu(W_gate), lay_gu(W_up), wd_prep


def prep_x_core(xc, h=H, tct=TC):
    nh0 = h // 128
    xt_c = np.ascontiguousarray(
        xc.astype(np.float16).T.reshape(nh0, 128, tct).transpose(1, 0, 2))
    return xt_c.reshape(128, nh0 * tct)


def run(inputs, trace=False, **kw):
    from concourse.bass_utils import run_bass_kernel_spmd

    if "nc" not in _CACHE:
        _CACHE["nc"] = build_nc()
    nc = _CACHE["nc"]

    wg_prep, wu_prep, wd_prep = prep_weights(
        inputs["W_gate"], inputs["W_up"], inputs["W_down"], inputs["active_idx"])
    x = inputs["x"]
    in_maps = [
        {"xt": prep_x_core(x[c * TC:(c + 1) * TC]),
         "wg": wg_prep, "wu": wu_prep, "wd": wd_prep}
        for c in range(NCORES)
    ]
    res = run_bass_kernel_spmd(nc, in_maps, core_ids=list(range(NCORES)),
                               trace=trace, **kw)
    out = np.concatenate([res.results[c]["out"] for c in range(NCORES)], axis=0)
    return out, res


def kernel(**inputs):
    out, _ = run(inputs, trace=False)
    return out


# revision 3
# speedup vs baseline: 1.2180x; 1.0213x over previous
"""LlamaSkipMLP Trainium2 kernel.

Strategy: data-parallel over the token dim across 8 NeuronCores (no
collectives).  Each core computes out_c = silu(x_c@Wg'.T) * (x_c@Wu'.T) @ Wd'.T
for its 1024-token slice, where Wg'/Wu'/Wd' are the active-neuron
gather of the weights (done host-side; for active_idx = arange(k) it
is a plain slice).

Device kernel (per core, Tile framework):
  phase 1: g/u GEMMs contract hidden dim H on the PE partitions.  The
           last two h-blocks (256 of 4096 contraction rows) run as a
           single fp8e4 DoubleRow matmul (2 MACs/cell) that opens each
           PSUM group; the remaining 30 h-blocks run in fp16.  The
           fp8 share is sized so the end-to-end relative error stays
           ~1.4e-2, under the 2e-2 gate.  Gate and up matmuls are
           interleaved within one h0 sweep per k0 so the x^T DMA only
           has to sustain ~300GB/s during k0=0.  SiLU on ACT, h =
           silu*up on DVE, h stored [k_part, t_free] in fp16.
  phase 2: down GEMM contracts the active-neuron dim k; h tiles are
           the stationary operand, W_down^T tiles the moving operand,
           so the output lands as [t_part, h_free] and stores
           contiguously.  The last hf block runs t1-outer/k0-inner
           against SBUF-resident wd tiles so its 8 PSUM groups finish
           staggered and the final drain+store tail is ~2us.

Scales: the fp8 pair computes (16*W)@(x/16) so the PSUM contribution
needs no correction.  PSUM accumulates fp32 throughout.
"""

import numpy as np

# Problem shapes (hardcoded per spec).
T, H, K = 8192, 4096, 3302
NCORES = 8
KP = 3328                 # K padded to a multiple of 128
NK0 = KP // 128           # 26 k-tiles
NH0 = H // 128            # 32 h-tiles (contraction, phase 1)
NH16 = NH0 - 2            # 30 h-tiles in fp16; last 2 via fp8 DoubleRow
TC = T // NCORES          # 1024 tokens per core
FP8_SCALE = 16.0

_CACHE = {}


def build_nc(kp=KP, h=H, tct=TC, enable_asserts=False):
    """Build + compile the per-core Bass program (SPMD: same on all cores)."""
    from contextlib import ExitStack

    import concourse.mybir as mybir
    import concourse.tile as tile
    from concourse import bacc

    fp16 = mybir.dt.float16
    fp32 = mybir.dt.float32
    fp8 = mybir.dt.float8e4
    DR = mybir.MatmulPerfMode.DoubleRow
    Silu = mybir.ActivationFunctionType.Silu
    Copy = mybir.ActivationFunctionType.Copy

    nk0 = kp // 128
    nh16 = NH16
    ntf = tct // 512          # moving t-tiles, phase 1 (2)
    nt1 = tct // 128          # stationary t-tiles, phase 2 (8)
    nhf = h // 512            # moving h-tiles, phase 2 (8)

    nc = bacc.Bacc(
        "TRN2", target_bir_lowering=False, debug=False,
        enable_asserts=enable_asserts,
    )
    xt = nc.dram_tensor("xt", [128, nh16 * tct], fp16, kind="ExternalInput").ap()
    xt8 = nc.dram_tensor("xt8", [128, 2, tct], fp8, kind="ExternalInput").ap()
    wg = nc.dram_tensor("wg", [nk0, 128, nh16 * 128], fp16, kind="ExternalInput").ap()
    wu = nc.dram_tensor("wu", [nk0, 128, nh16 * 128], fp16, kind="ExternalInput").ap()
    wg8 = nc.dram_tensor("wg8", [nk0, 128, 2, 128], fp8, kind="ExternalInput").ap()
    wu8 = nc.dram_tensor("wu8", [nk0, 128, 2, 128], fp8, kind="ExternalInput").ap()
    wd = nc.dram_tensor("wd", [nk0, 128, h], fp16, kind="ExternalInput").ap()
    out = nc.dram_tensor("out", [tct, h], fp32, kind="ExternalOutput").ap()

    with tile.TileContext(nc) as tc, ExitStack() as ctx:
        h_pool = ctx.enter_context(tc.tile_pool(name="hp", bufs=1))
        w_pool = ctx.enter_context(tc.tile_pool(name="wp", bufs=2))
        w8_pool = ctx.enter_context(tc.tile_pool(name="w8p", bufs=2))
        tmp_pool = ctx.enter_context(tc.tile_pool(name="tmpp", bufs=2))
        out_pool = ctx.enter_context(tc.tile_pool(name="outp", bufs=8))
        wd7_pool = ctx.enter_context(tc.tile_pool(name="wd7p", bufs=nk0))
        wd_pool = ctx.enter_context(tc.tile_pool(name="wdp", bufs=8))
        xt_pool = ctx.enter_context(tc.tile_pool(name="xtp", bufs=1))

        xt_sb = xt_pool.tile([128, nh16 * tct], fp16, name="xt_sb")
        xt8_sb = xt_pool.tile([128, 2, tct], fp8, name="xt8_sb", tag="xt8")
        h_sb = h_pool.tile([128, nk0 * tct], fp16, name="h_sb")

        wd7_t = [wd7_pool.tile([128, 512], fp16, name=f"wd7_{k}", tag="wd7")
                 for k in range(nk0)]

        # Startup: tiny fp8 pieces first (the DoubleRow matmuls open each
        # PSUM group), then the first fp16 weight/x pieces, then the rest.
        wg8_t0 = w8_pool.tile([128, 2, 128], fp8, name="wg8_t", tag="wg8")
        wu8_t0 = w8_pool.tile([128, 2, 128], fp8, name="wu8_t", tag="wu8")
        nc.sync.dma_start(wg8_t0[:, :, :], wg8[0])
        nc.sync.dma_start(wu8_t0[:, :, :], wu8[0])
        nc.sync.dma_start(xt8_sb[:, :, 0:512], xt8[:, :, 0:512])
        nc.sync.dma_start(xt8_sb[:, :, 512:1024], xt8[:, :, 512:1024])

        wg_t0 = w_pool.tile([128, nh16 * 128], fp16, name="wg_t", tag="wg")
        wu_t0 = w_pool.tile([128, nh16 * 128], fp16, name="wu_t", tag="wu")
        # First fp16 pieces small (h0=0,1), then the rest of the slabs.
        nc.sync.dma_start(wg_t0[:, 0:256], wg[0, :, 0:256])
        nc.sync.dma_start(wu_t0[:, 0:256], wu[0, :, 0:256])
        nc.sync.dma_start(xt_sb[:, 0:512], xt[:, 0:512])
        nc.sync.dma_start(xt_sb[:, 512:1024], xt[:, 512:1024])
        nc.sync.dma_start(xt_sb[:, 1024:2048], xt[:, 1024:2048])
        wcsz = 1024
        for i in range(1, 4):
            nc.sync.dma_start(wg_t0[:, i * wcsz - 768:(i + 1) * wcsz - 768],
                              wg[0, :, i * wcsz - 768:(i + 1) * wcsz - 768])
            nc.sync.dma_start(wu_t0[:, i * wcsz - 768:(i + 1) * wcsz - 768],
                              wu[0, :, i * wcsz - 768:(i + 1) * wcsz - 768])
            nc.sync.dma_start(xt_sb[:, (i + 1) * wcsz:(i + 2) * wcsz],
                              xt[:, (i + 1) * wcsz:(i + 2) * wcsz])
        nc.sync.dma_start(wg_t0[:, 4 * wcsz - 768:nh16 * 128],
                          wg[0, :, 4 * wcsz - 768:nh16 * 128])
        nc.sync.dma_start(wu_t0[:, 4 * wcsz - 768:nh16 * 128],
                          wu[0, :, 4 * wcsz - 768:nh16 * 128])
        for i in range(5, nh16):
            nc.sync.dma_start(xt_sb[:, i * 1024:(i + 1) * 1024],
                              xt[:, i * 1024:(i + 1) * 1024])

        # ---- phase 1: g = x@Wg^T, u = x@Wu^T, h = silu(g)*u ----
        with tc.tile_pool(name="ps1", space="PSUM", bufs=2) as ps1:
            for k0 in range(nk0):
                if k0 == 0:
                    wg_t, wu_t = wg_t0, wu_t0
                    wg8_t, wu8_t = wg8_t0, wu8_t0
                else:
                    wg_t = w_pool.tile([128, nh16 * 128], fp16, name="wg_t", tag="wg")
                    nc.sync.dma_start(wg_t[:, :], wg[k0])
                    wu_t = w_pool.tile([128, nh16 * 128], fp16, name="wu_t", tag="wu")
                    nc.sync.dma_start(wu_t[:, :], wu[k0])
                    wg8_t = w8_pool.tile([128, 2, 128], fp8, name="wg8_t", tag="wg8")
                    nc.sync.dma_start(wg8_t[:, :, :], wg8[k0])
                    wu8_t = w8_pool.tile([128, 2, 128], fp8, name="wu8_t", tag="wu8")
                    nc.sync.dma_start(wu8_t[:, :, :], wu8[k0])
                if k0 >= 1:
                    # Spread the resident last-hf wd loads across phase 1.
                    nc.sync.dma_start(wd7_t[k0 - 1][:, :],
                                      wd[k0 - 1, :, (nhf - 1) * 512:nhf * 512])
                    if k0 == nk0 - 1:
                        nc.sync.dma_start(wd7_t[k0][:, :],
                                          wd[k0, :, (nhf - 1) * 512:nhf * 512])
                pg = [ps1.tile([128, 512], fp32, name=f"pg{i}", tag=f"pg{i}")
                      for i in range(ntf)]
                pu = [ps1.tile([128, 512], fp32, name=f"pu{i}", tag=f"pu{i}")
                      for i in range(ntf)]
                # fp8 DoubleRow pair opens each PSUM group (256 rows).
                for i in range(ntf):
                    nc.tensor.matmul(
                        pg[i][:, :], wg8_t[:, :, :],
                        xt8_sb[:, :, i * 512:(i + 1) * 512],
                        start=True, stop=False, perf_mode=DR,
                    )
                    nc.tensor.matmul(
                        pu[i][:, :], wu8_t[:, :, :],
                        xt8_sb[:, :, i * 512:(i + 1) * 512],
                        start=True, stop=False, perf_mode=DR,
                    )
                # Interleave gate/up so each x chunk feeds 4 matmuls.
                for h0 in range(nh16):
                    for i in range(ntf):
                        nc.tensor.matmul(
                            pg[i][:, :], wg_t[:, h0 * 128:(h0 + 1) * 128],
                            xt_sb[:, h0 * tct + i * 512:h0 * tct + (i + 1) * 512],
                            start=False, stop=(h0 == nh16 - 1),
                        )
                        nc.tensor.matmul(
                            pu[i][:, :], wu_t[:, h0 * 128:(h0 + 1) * 128],
                            xt_sb[:, h0 * tct + i * 512:h0 * tct + (i + 1) * 512],
                            start=False, stop=(h0 == nh16 - 1),
                        )
                for i in range(ntf):
                    sg = tmp_pool.tile([128, 512], fp32, name="sg", tag="sg")
                    nc.scalar.activation(sg[:, :], pg[i][:, :], Silu)
                    nc.vector.tensor_mul(
                        h_sb[:, k0 * tct + i * 512:k0 * tct + (i + 1) * 512],
                        sg[:, :], pu[i][:, :])

        # ---- phase 2: out = h @ Wd^T (contract k) ----
        with tc.tile_pool(name="ps2", space="PSUM", bufs=1) as ps2:
            for hf in range(nhf - 1):
                po = [ps2.tile([128, 512], fp32, name=f"po{t1}", tag=f"po{t1}")
                      for t1 in range(nt1)]
                for k0 in range(nk0):
                    wd_t = wd_pool.tile([128, 512], fp16, name="wd_t", tag="wd")
                    nc.sync.dma_start(wd_t[:, :], wd[k0, :, hf * 512:(hf + 1) * 512])
                    for t1 in range(nt1):
                        nc.tensor.matmul(
                            po[t1][:, :],
                            h_sb[:, k0 * tct + t1 * 128:k0 * tct + (t1 + 1) * 128],
                            wd_t[:, :],
                            start=(k0 == 0), stop=(k0 == nk0 - 1),
                        )
                # Drains alternate DVE / ACT so the two engines empty the
                # PSUM banks in parallel and the next hf's matmuls don't
                # stall on bank reuse.
                for t1 in range(nt1):
                    ot = out_pool.tile([128, 512], fp32, name="ot", tag="ot")
                    if t1 % 2 == 0:
                        nc.vector.tensor_copy(ot[:, :], po[t1][:, :])
                    else:
                        nc.scalar.activation(ot[:, :], po[t1][:, :], Copy)
                    nc.sync.dma_start(
                        out[t1 * 128:(t1 + 1) * 128, hf * 512:(hf + 1) * 512],
                        ot[:, :])
            # Last hf: t1-outer / k0-inner against resident wd tiles, so
            # each PSUM group completes 26 matmuls before the next starts
            # and drains+stores overlap the remaining matmuls.
            hf = nhf - 1
            for t1 in range(nt1):
                po = ps2.tile([128, 512], fp32, name=f"po{t1}", tag=f"po{t1}")
                for k0 in range(nk0):
                    nc.tensor.matmul(
                        po[:, :],
                        h_sb[:, k0 * tct + t1 * 128:k0 * tct + (t1 + 1) * 128],
                        wd7_t[k0][:, :],
                        start=(k0 == 0), stop=(k0 == nk0 - 1),
                    )
                ot = out_pool.tile([128, 512], fp32, name="ot", tag="ot")
                if t1 % 2 == 0:
                    nc.vector.tensor_copy(ot[:, :], po[:, :])
                else:
                    nc.scalar.activation(ot[:, :], po[:, :], Copy)
                nc.sync.dma_start(
                    out[t1 * 128:(t1 + 1) * 128, hf * 512:(hf + 1) * 512],
                    ot[:, :])

    nc.compile()
    return nc


def prep_weights(W_gate, W_up, W_down, active_idx, kp=KP, h=H):
    import ml_dtypes
    idx = np.asarray(active_idx)
    k = idx.shape[0]
    nk0 = kp // 128
    nh16 = NH16
    hc = nh16 * 128

    def lay_gu(W):
        a = np.zeros((kp, h), np.float32)
        a[:k] = W[idx]
        lo = np.ascontiguousarray(
            a[:, :hc].astype(np.float16)
            .reshape(nk0, 128, nh16, 128).transpose(0, 3, 2, 1)
        ).reshape(nk0, 128, nh16 * 128)
        # fp8 pair: [k0, p, i, c] = 16*W[k0*128+c, hc + i*128 + p]
        w8 = np.clip(a[:, hc:] * FP8_SCALE, -240, 240)
        w8 = np.ascontiguousarray(
            w8.reshape(nk0, 128, 2, 128).transpose(0, 3, 2, 1)
        ).astype(ml_dtypes.float8_e4m3)
        return lo, w8

    wg_prep, wg8_prep = lay_gu(W_gate)
    wu_prep, wu8_prep = lay_gu(W_up)
    wd_a = np.zeros((kp, h), np.float16)
    wd_a[:k] = W_down[:, idx].T.astype(np.float16)
    wd_prep = np.ascontiguousarray(wd_a.reshape(nk0, 128, h))
    return wg_prep, wg8_prep, wu_prep, wu8_prep, wd_prep


def prep_x_core(xc, h=H, tct=TC):
    import ml_dtypes
    nh16 = NH16
    hc = nh16 * 128
    xt_c = np.ascontiguousarray(
        xc[:, :hc].astype(np.float16).T.reshape(nh16, 128, tct).transpose(1, 0, 2))
    x8 = np.ascontiguousarray(
        (xc[:, hc:].astype(np.float32) / FP8_SCALE)
        .T.reshape(2, 128, tct).transpose(1, 0, 2)
    ).astype(ml_dtypes.float8_e4m3)
    return xt_c.reshape(128, nh16 * tct), x8


def run(inputs, trace=False, **kw):
    from concourse.bass_utils import run_bass_kernel_spmd

    if "nc" not in _CACHE:
        _CACHE["nc"] = build_nc()
    nc = _CACHE["nc"]

    wg_prep, wg8_prep, wu_prep, wu8_prep, wd_prep = prep_weights(
        inputs["W_gate"], inputs["W_up"], inputs["W_down"], inputs["active_idx"])
    x = inputs["x"]
    in_maps = []
    for c in range(NCORES):
        xt_c, x8_c = prep_x_core(x[c * TC:(c + 1) * TC])
        in_maps.append({"xt": xt_c, "xt8": x8_c, "wg": wg_prep, "wg8": wg8_prep,
                        "wu": wu_prep, "wu8": wu8_prep, "wd": wd_prep})
    res = run_bass_kernel_spmd(nc, in_maps, core_ids=list(range(NCORES)),
                               trace=trace, **kw)
    out = np.concatenate([res.results[c]["out"] for c in range(NCORES)], axis=0)
    return out, res


def kernel(**inputs):
    out, _ = run(inputs, trace=False)
    return out


# revision 5
# speedup vs baseline: 1.2414x; 1.0192x over previous
"""LlamaSkipMLP Trainium2 kernel.

Strategy: data-parallel over the token dim across 8 NeuronCores (no
collectives).  Each core computes out_c = silu(x_c@Wg'.T) * (x_c@Wu'.T) @ Wd'.T
for its 1024-token slice, where Wg'/Wu'/Wd' are the active-neuron
gather of the weights (done host-side; for active_idx = arange(k) it
is a plain slice).

Device kernel (per core, Tile framework):
  phase 1: g/u GEMMs contract hidden dim H on the PE partitions.  The
           last four h-blocks (512 of 4096 contraction rows) run as
           two fp8e4 DoubleRow matmuls (2 MACs/cell) that open each
           PSUM group; the remaining 28 h-blocks run in fp16.  The
           fp8 share is sized so the end-to-end relative error stays
           ~1.6e-2, under the 2e-2 gate.  Gate/up matmuls interleave
           within one h0 sweep, and k0=0,1 run as one interleaved
           sweep, so the x^T DMA only has to sustain ~180GB/s at
           kernel start instead of ~600GB/s.  SiLU on ACT, h=silu*up
           on DVE, h stored [k_part, t_free] in fp16.
  phase 2: down GEMM contracts the active-neuron dim k; h tiles are
           the stationary operand, W_down^T tiles the moving operand,
           so the output lands as [t_part, h_free] and stores
           contiguously.  The last hf block runs t1-outer/k0-inner
           against SBUF-resident wd tiles so its 8 PSUM groups finish
           staggered and the final drain+store tail is ~2us.

Scales: the fp8 blocks compute (16*W)@(x/16) so the PSUM contribution
needs no correction.  PSUM accumulates fp32 throughout.
"""

import numpy as np

# Problem shapes (hardcoded per spec).
T, H, K = 8192, 4096, 3302
NCORES = 8
KP = 3328                 # K padded to a multiple of 128
NK0 = KP // 128           # 26 k-tiles
NH0 = H // 128            # 32 h-tiles (contraction, phase 1)
NP8 = 2                   # fp8 DoubleRow pairs (2 h-blocks each)
NH16 = NH0 - 2 * NP8      # 28 h-tiles in fp16
TC = T // NCORES          # 1024 tokens per core
FP8_SCALE = 16.0

_CACHE = {}


def build_nc(kp=KP, h=H, tct=TC, enable_asserts=False):
    """Build + compile the per-core Bass program (SPMD: same on all cores)."""
    from contextlib import ExitStack

    import concourse.mybir as mybir
    import concourse.tile as tile
    from concourse import bacc

    fp16 = mybir.dt.float16
    fp32 = mybir.dt.float32
    fp8 = mybir.dt.float8e4
    DR = mybir.MatmulPerfMode.DoubleRow
    Silu = mybir.ActivationFunctionType.Silu
    Copy = mybir.ActivationFunctionType.Copy

    nk0 = kp // 128
    nh16 = NH16
    np8 = NP8
    ntf = tct // 512          # moving t-tiles, phase 1 (2)
    nt1 = tct // 128          # stationary t-tiles, phase 2 (8)
    nhf = h // 512            # moving h-tiles, phase 2 (8)

    nc = bacc.Bacc(
        "TRN2", target_bir_lowering=False, debug=False,
        enable_asserts=enable_asserts,
    )
    xt = nc.dram_tensor("xt", [128, nh16 * tct], fp16, kind="ExternalInput").ap()
    xt8 = nc.dram_tensor("xt8", [2 * np8, 128, tct], fp8, kind="ExternalInput").ap()
    wg = nc.dram_tensor("wg", [nk0, 128, nh16 * 128], fp16, kind="ExternalInput").ap()
    wu = nc.dram_tensor("wu", [nk0, 128, nh16 * 128], fp16, kind="ExternalInput").ap()
    wg8 = nc.dram_tensor("wg8", [nk0, 128, 2 * np8, 128], fp8,
                         kind="ExternalInput").ap()
    wu8 = nc.dram_tensor("wu8", [nk0, 128, 2 * np8, 128], fp8,
                         kind="ExternalInput").ap()
    wd = nc.dram_tensor("wd", [nk0, 128, h], fp16, kind="ExternalInput").ap()
    out = nc.dram_tensor("out", [tct, h], fp32, kind="ExternalOutput").ap()

    with tile.TileContext(nc) as tc, ExitStack() as ctx:
        h_pool = ctx.enter_context(tc.tile_pool(name="hp", bufs=1))
        w_pool = ctx.enter_context(tc.tile_pool(name="wp", bufs=3))
        w8_pool = ctx.enter_context(tc.tile_pool(name="w8p", bufs=3))
        tmp_pool = ctx.enter_context(tc.tile_pool(name="tmpp", bufs=2))
        out_pool = ctx.enter_context(tc.tile_pool(name="outp", bufs=8))
        wd7_pool = ctx.enter_context(tc.tile_pool(name="wd7p", bufs=nk0))
        wd_pool = ctx.enter_context(tc.tile_pool(name="wdp", bufs=6))
        xt_pool = ctx.enter_context(tc.tile_pool(name="xtp", bufs=1))

        xt_sb = xt_pool.tile([128, nh16 * tct], fp16, name="xt_sb")
        xt8_sb = xt_pool.tile([128, 2 * np8, tct], fp8, name="xt8_sb", tag="xt8")
        h_sb = h_pool.tile([128, nk0 * tct], fp16, name="h_sb")

        wd7_t = [wd7_pool.tile([128, 512], fp16, name=f"wd7_{k}", tag="wd7")
                 for k in range(nk0)]

        def load_w16(k0):
            wg_t = w_pool.tile([128, nh16 * 128], fp16, name="wg_t", tag="wg")
            nc.sync.dma_start(wg_t[:, :], wg[k0])
            wu_t = w_pool.tile([128, nh16 * 128], fp16, name="wu_t", tag="wu")
            nc.sync.dma_start(wu_t[:, :], wu[k0])
            return wg_t, wu_t

        def load_w8(k0):
            wg8_t = w8_pool.tile([128, 2 * np8, 128], fp8, name="wg8_t", tag="wg8")
            nc.sync.dma_start(wg8_t[:, :, :], wg8[k0])
            wu8_t = w8_pool.tile([128, 2 * np8, 128], fp8, name="wu8_t", tag="wu8")
            nc.sync.dma_start(wu8_t[:, :, :], wu8[k0])
            return wg8_t, wu8_t

        # --- startup DMA schedule ---
        # Weights issue from the Sync HWDGE; all x chunks issue from the
        # ACT HWDGE (idle at startup) so the two streams don't serialize
        # on one engine's ~650ns-per-DMA issue rate.
        wg8_t0, wu8_t0 = load_w8(0)
        wg8_t1, wu8_t1 = load_w8(1)
        wg_t0 = w_pool.tile([128, nh16 * 128], fp16, name="wg_t", tag="wg")
        wu_t0 = w_pool.tile([128, nh16 * 128], fp16, name="wu_t", tag="wu")
        wg_t1 = w_pool.tile([128, nh16 * 128], fp16, name="wg_t", tag="wg")
        wu_t1 = w_pool.tile([128, nh16 * 128], fp16, name="wu_t", tag="wu")
        wpieces = [(0, 512), (512, 1536), (1536, 2560), (2560, nh16 * 128)]
        for a, b in wpieces:
            for wt, wsrc, k0 in ((wg_t0, wg, 0), (wu_t0, wu, 0),
                                 (wg_t1, wg, 1), (wu_t1, wu, 1)):
                nc.sync.dma_start(wt[:, a:b], wsrc[k0, :, a:b])
        # ACT queue: xt8 chunks (64KB) interleaved with leading xt chunks,
        # in matmul consumption order.
        def xt8_chunk(jj, tt):
            nc.scalar.dma_start(xt8_sb[:, jj, tt * 512:(tt + 1) * 512],
                                xt8[jj, :, tt * 512:(tt + 1) * 512])
        def xt_chunk(a, b):
            nc.scalar.dma_start(xt_sb[:, a:b], xt[:, a:b])
        xt8_chunk(0, 0); xt8_chunk(1, 0)
        xt_chunk(0, 512); xt_chunk(512, 1024)
        xt8_chunk(2, 0); xt8_chunk(3, 0)
        xt_chunk(1024, 1536); xt_chunk(1536, 2048)
        xt8_chunk(0, 1); xt8_chunk(1, 1)
        xt_chunk(2048, 2560); xt_chunk(2560, 3072)
        xt8_chunk(2, 1); xt8_chunk(3, 1)
        xt_chunk(3072, 3584); xt_chunk(3584, 4096)
        for i in range(4, nh16):
            xt_chunk(i * 1024, (i + 1) * 1024)

        def dr_mms(pg, pu, wg8_t, wu8_t, i, j):
            nc.tensor.matmul(
                pg[i][:, :], wg8_t[:, 2 * j:2 * j + 2, :],
                xt8_sb[:, 2 * j:2 * j + 2, i * 512:(i + 1) * 512],
                start=(j == 0), stop=False, perf_mode=DR,
            )
            nc.tensor.matmul(
                pu[i][:, :], wu8_t[:, 2 * j:2 * j + 2, :],
                xt8_sb[:, 2 * j:2 * j + 2, i * 512:(i + 1) * 512],
                start=(j == 0), stop=False, perf_mode=DR,
            )

        def f16_mms(pg, pu, wg_t, wu_t, h0, i):
            nc.tensor.matmul(
                pg[i][:, :], wg_t[:, h0 * 128:(h0 + 1) * 128],
                xt_sb[:, h0 * tct + i * 512:h0 * tct + (i + 1) * 512],
                start=False, stop=(h0 == nh16 - 1),
            )
            nc.tensor.matmul(
                pu[i][:, :], wu_t[:, h0 * 128:(h0 + 1) * 128],
                xt_sb[:, h0 * tct + i * 512:h0 * tct + (i + 1) * 512],
                start=False, stop=(h0 == nh16 - 1),
            )

        def drain(pg, pu, k0):
            for i in range(ntf):
                sg = tmp_pool.tile([128, 512], fp16, name="sg", tag="sg")
                nc.scalar.activation(sg[:, :], pg[i][:, :], Silu)
                nc.vector.tensor_mul(
                    h_sb[:, k0 * tct + i * 512:k0 * tct + (i + 1) * 512],
                    sg[:, :], pu[i][:, :])

        # ---- phase 1: g = x@Wg^T, u = x@Wu^T, h = silu(g)*u ----
        with tc.tile_pool(name="ps1", space="PSUM", bufs=2) as ps1:
            def ptiles():
                pg = [ps1.tile([128, 512], fp32, name=f"pg{i}", tag=f"pg{i}")
                      for i in range(ntf)]
                pu = [ps1.tile([128, 512], fp32, name=f"pu{i}", tag=f"pu{i}")
                      for i in range(ntf)]
                return pg, pu

            # k0 = 0,1 as one interleaved sweep (uses all 8 PSUM banks);
            # halves the startup x^T bandwidth demand.
            pp = [ptiles(), ptiles()]
            ww = [(wg_t0, wu_t0, wg8_t0, wu8_t0), (wg_t1, wu_t1, wg8_t1, wu8_t1)]
            for i in range(ntf):
                for j in range(np8):
                    for kk in (0, 1):
                        dr_mms(pp[kk][0], pp[kk][1], ww[kk][2], ww[kk][3], i, j)
            for h0 in range(nh16):
                for i in range(ntf):
                    for kk in (0, 1):
                        f16_mms(pp[kk][0], pp[kk][1], ww[kk][0], ww[kk][1], h0, i)
            for kk in (0, 1):
                drain(pp[kk][0], pp[kk][1], kk)

            for k0 in range(2, nk0):
                wg_t, wu_t = load_w16(k0)
                wg8_t, wu8_t = load_w8(k0)
                # Spread the resident last-hf wd loads across phase 1.
                nc.sync.dma_start(wd7_t[k0 - 2][:, :],
                                  wd[k0 - 2, :, (nhf - 1) * 512:nhf * 512])
                if k0 == nk0 - 1:
                    for kk in (nk0 - 2, nk0 - 1):
                        nc.sync.dma_start(wd7_t[kk][:, :],
                                          wd[kk, :, (nhf - 1) * 512:nhf * 512])
                pg, pu = ptiles()
                for i in range(ntf):
                    for j in range(np8):
                        dr_mms(pg, pu, wg8_t, wu8_t, i, j)
                for h0 in range(nh16):
                    for i in range(ntf):
                        f16_mms(pg, pu, wg_t, wu_t, h0, i)
                drain(pg, pu, k0)

        # ---- phase 2: out = h @ Wd^T (contract k) ----
        with tc.tile_pool(name="ps2", space="PSUM", bufs=1) as ps2:
            for hf in range(nhf - 1):
                po = [ps2.tile([128, 512], fp32, name=f"po{t1}", tag=f"po{t1}")
                      for t1 in range(nt1)]
                for k0 in range(nk0):
                    wd_t = wd_pool.tile([128, 512], fp16, name="wd_t", tag="wd")
                    nc.sync.dma_start(wd_t[:, :], wd[k0, :, hf * 512:(hf + 1) * 512])
                    for t1 in range(nt1):
                        nc.tensor.matmul(
                            po[t1][:, :],
                            h_sb[:, k0 * tct + t1 * 128:k0 * tct + (t1 + 1) * 128],
                            wd_t[:, :],
                            start=(k0 == 0), stop=(k0 == nk0 - 1),
                        )
                # Drains alternate DVE / ACT so the two engines empty the
                # PSUM banks in parallel and the next hf's matmuls don't
                # stall on bank reuse.
                for t1 in range(nt1):
                    ot = out_pool.tile([128, 512], fp32, name="ot", tag="ot")
                    if t1 % 2 == 0:
                        nc.vector.tensor_copy(ot[:, :], po[t1][:, :])
                    else:
                        nc.scalar.activation(ot[:, :], po[t1][:, :], Copy)
                    nc.sync.dma_start(
                        out[t1 * 128:(t1 + 1) * 128, hf * 512:(hf + 1) * 512],
                        ot[:, :])
            # Last hf: t1-outer / k0-inner against resident wd tiles, so
            # each PSUM group completes 26 matmuls before the next starts
            # and drains+stores overlap the remaining matmuls.
            hf = nhf - 1
            for t1 in range(nt1):
                po = ps2.tile([128, 512], fp32, name=f"po{t1}", tag=f"po{t1}")
                for k0 in range(nk0):
                    nc.tensor.matmul(
                        po[:, :],
                        h_sb[:, k0 * tct + t1 * 128:k0 * tct + (t1 + 1) * 128],
                        wd7_t[k0][:, :],
                        start=(k0 == 0), stop=(k0 == nk0 - 1),
                    )
                ot = out_pool.tile([128, 512], fp32, name="ot", tag="ot")
                if t1 % 2 == 0:
                    nc.vector.tensor_copy(ot[:, :], po[:, :])
                else:
                    nc.scalar.activation(ot[:, :], po[:, :], Copy)
                nc.sync.dma_start(
                    out[t1 * 128:(t1 + 1) * 128, hf * 512:(hf + 1) * 512],
                    ot[:, :])

    nc.compile()
    return nc


def prep_weights(W_gate, W_up, W_down, active_idx, kp=KP, h=H):
    import ml_dtypes
    idx = np.asarray(active_idx)
    k = idx.shape[0]
    nk0 = kp // 128
    nh16 = NH16
    nb8 = 2 * NP8
    hc = nh16 * 128

    def lay_gu(W):
        a = np.zeros((kp, h), np.float32)
        a[:k] = W[idx]
        lo = np.ascontiguousarray(
            a[:, :hc].astype(np.float16)
            .reshape(nk0, 128, nh16, 128).transpose(0, 3, 2, 1)
        ).reshape(nk0, 128, nh16 * 128)
        # fp8 blocks: [k0, p, jj, c] = 16*W[k0*128+c, hc + jj*128 + p]
        w8 = np.clip(a[:, hc:] * FP8_SCALE, -240, 240)
        w8 = np.ascontiguousarray(
            w8.reshape(nk0, 128, nb8, 128).transpose(0, 3, 2, 1)
        ).astype(ml_dtypes.float8_e4m3)
        return lo, w8

    wg_prep, wg8_prep = lay_gu(W_gate)
    wu_prep, wu8_prep = lay_gu(W_up)
    wd_a = np.zeros((kp, h), np.float16)
    wd_a[:k] = W_down[:, idx].T.astype(np.float16)
    wd_prep = np.ascontiguousarray(wd_a.reshape(nk0, 128, h))
    return wg_prep, wg8_prep, wu_prep, wu8_prep, wd_prep


def prep_x_core(xc, h=H, tct=TC):
    import ml_dtypes
    nh16 = NH16
    hc = nh16 * 128
    xt_c = np.ascontiguousarray(
        xc[:, :hc].astype(np.float16).T.reshape(nh16, 128, tct).transpose(1, 0, 2))
    # [jj, p, t] = x[t, hc + jj*128 + p] / 16
    x8 = np.ascontiguousarray(
        (xc[:, hc:].astype(np.float32) / FP8_SCALE).T.reshape(2 * NP8, 128, tct)
    ).astype(ml_dtypes.float8_e4m3)
    return xt_c.reshape(128, nh16 * tct), x8


def run(inputs, trace=False, **kw):
    from concourse.bass_utils import run_bass_kernel_spmd

    if "nc" not in _CACHE:
        _CACHE["nc"] = build_nc()
    nc = _CACHE["nc"]

    wg_prep, wg8_prep, wu_prep, wu8_prep, wd_prep = prep_weights(
        inputs["W_gate"], inputs["W_up"], inputs["W_down"], inputs["active_idx"])
    x = inputs["x"]
    in_maps = []
    for c in range(NCORES):
        xt_c, x8_c = prep_x_core(x[c * TC:(c + 1) * TC])
        in_maps.append({"xt": xt_c, "xt8": x8_c, "wg": wg_prep, "wg8": wg8_prep,
                        "wu": wu_prep, "wu8": wu8_prep, "wd": wd_prep})
    res = run_bass_kernel_spmd(nc, in_maps, core_ids=list(range(NCORES)),
                               trace=trace, **kw)
    out = np.concatenate([res.results[c]["out"] for c in range(NCORES)], axis=0)
    return out, res


def kernel(**inputs):
    out, _ = run(inputs, trace=False)
    return out
